# revision 15
# baseline (speedup 1.0000x reference)
"""DGCN link prediction on 8 Trainium2 NeuronCores.

Layout spec (per core c, edge set k, window w):
  - window = 16 bins x 32 psum cols = 512 cols; bin jj covers cols
    [32*jj, 32*jj+32) of the window, i.e. padded node ids
    c*PADN + w*512 + 32*jj + rank.
  - gather buffer g_sb = 64 blocks of 128 slots; block b = q*16 + jj holds
    the chunk-q slots of bin jj (q in [0,4)).
  - gather call (k, w, q) has 2048 indices; index position i = jj*128 + s
    maps to block jj of that call's quarter -> g partition s, block q*16+jj.
  - S matrix s_sb [128 (slot s), 64*32]: S[s, (q*16+jj)*32 + rank] = norm of
    slot (q, jj, s); the 4 chunk-tiles of bin jj accumulate into the same
    psum cols [32*jj, 32*jj+32) via matmul start=(q==0).
Table rows are 128 bf16 (256B) with cols 0:64 live; chunk q of the table is
rows [q*CHUNK, (q+1)*CHUNK), CHUNK = 2*PADN <= 32767 (int16 gather indices).
"""
import sys, os
sys.path.insert(0, '/opt/trn_rl_repo')
if '/root/.axon_site' not in sys.path:
    sys.path.insert(0, '/root/.axon_site')
import numpy as np
import ml_dtypes

BF16 = ml_dtypes.bfloat16

N = 100000
E = 1600000
F_IN = 128
H = 64
Q = 10000
CORES = 8
NPC = N // CORES
BIN_COLS = 64
BINS_PER_WIN = 8
TILES_PER_BINCHUNK = 2
BIN_CAP = 128 * TILES_PER_BINCHUNK
WIN_COLS = BIN_COLS * BINS_PER_WIN   # 512
NCHUNK = 4
QPC = Q // CORES                     # queries per core
QT = -(-QPC // 128)                  # query tiles per core (of 128)

LAST_HW_NS = None


def _norms(row, col, w):
    deg = np.bincount(col, weights=w.astype(np.float64), minlength=N) + 1.0
    dinv = (1.0 / np.sqrt(deg)).astype(np.float32)
    norm = dinv[row] * w.astype(np.float32) * dinv[col]
    return norm, dinv


def _split128(a, b):
    """split [a,b) at multiples of 128"""
    out = []
    while a < b:
        e = min(b, (a // 128 + 1) * 128)
        out.append((a, e))
        a = e
    return out


class Packing:
    pass


def pack(inputs):
    P = Packing()
    sets = []
    for key, wkey in (("edge_index", None), ("edge_in", "in_w"), ("edge_out", "out_w")):
        ei = np.asarray(inputs[key]).astype(np.int64)
        row, col = ei[0].astype(np.int32), ei[1].astype(np.int32)
        w = (np.ones(E, np.float32) if wkey is None
             else np.asarray(inputs[wkey]).astype(np.float32))
        norm, dinv = _norms(row, col, w)
        sets.append((row, col, norm, dinv))

    chunk_of = (np.arange(N, dtype=np.int32) // NPC) // 2
    cnt = np.zeros((3, N, NCHUNK), np.int32)
    for k, (row, col, norm, dinv) in enumerate(sets):
        np.add.at(cnt[k], (col, chunk_of[row]), 1)
        cnt[k][np.arange(N), chunk_of] += 1  # self loop

    # ---- bin packing: first-fit over K_open open bins, per core ----
    # bins_per_core[c] = list of np arrays of node ids (arbitrary membership)
    bins_per_core = []
    cntT = cnt.transpose(1, 0, 2).reshape(N, 3 * NCHUNK).astype(np.int64)  # [N, 12]
    K_OPEN = 32
    for c in range(CORES):
        closed = []
        open_cnt = np.zeros((K_OPEN, 12), np.int64)
        open_len = np.zeros(K_OPEN, np.int64)
        open_nodes = [[] for _ in range(K_OPEN)]
        core_nodes = np.arange(c * NPC, (c + 1) * NPC)
        order_ffd = core_nodes[np.argsort(-cntT[core_nodes].max(axis=1), kind="stable")]
        for n in order_ffd:
            nc_ = cntT[n]
            newc = open_cnt + nc_
            fits = (open_len < BIN_COLS) & (newc <= BIN_CAP).all(axis=1)
            if fits.any():
                # best fit: balance the binding constraint across bins
                score = np.where(fits, newc.max(axis=1), 10**9)
                j = int(np.argmin(score))
            else:
                # close the fullest open bin, reuse its slot
                j = int(np.argmax([open_cnt[i].max() for i in range(K_OPEN)]))
                closed.append(np.array(open_nodes[j], np.int32))
                open_nodes[j] = []
                open_cnt[j] = 0
                open_len[j] = 0
            open_nodes[j].append(n)
            open_cnt[j] += nc_
            open_len[j] += 1
        for j in range(K_OPEN):
            if open_nodes[j]:
                closed.append(np.array(open_nodes[j], np.int32))
        bins_per_core.append(closed)

    nbins_max = max(len(b) for b in bins_per_core)
    NW = -(-nbins_max // BINS_PER_WIN)
    PADN = NW * WIN_COLS
    CHUNK = 2 * PADN
    assert CHUNK <= 32768, f"CHUNK={CHUNK} too big (NW={NW})"  # max idx = CHUNK-1
    TROWS = CORES * PADN

    pid = np.full(N, -1, np.int64)
    for c in range(CORES):
        for bi, bnodes in enumerate(bins_per_core[c]):
            base = c * PADN + bi * BIN_COLS
            pid[bnodes] = base + np.arange(len(bnodes))
    assert (pid >= 0).all()

    idx_all = np.zeros((CORES, 3, NW, NCHUNK, 2048), np.int32)
    s_all = np.zeros((CORES, 3, NW, 128, 64 * BIN_COLS), np.float32)  # [128, 4096]

    for k, (row, col, norm, dinv) in enumerate(sets):
        loop_src = np.arange(N, dtype=np.int32)
        r_full = np.concatenate([row, loop_src])
        c_full = np.concatenate([col, loop_src])
        n_full = np.concatenate([norm, dinv * dinv])
        q_full = chunk_of[r_full]
        order = np.lexsort((q_full, c_full))
        r_s, n_s, q_s = r_full[order], n_full[order], q_full[order]
        c_s = c_full[order]
        flat_cnt = np.bincount(c_s.astype(np.int64) * NCHUNK + q_s,
                               minlength=N * NCHUNK).reshape(N, NCHUNK)
        starts = np.zeros((N, NCHUNK + 1), np.int64)
        np.cumsum(flat_cnt, axis=1, out=starts[:, 1:])
        node_base = np.zeros(N + 1, np.int64)
        np.cumsum(flat_cnt.sum(1), out=node_base[1:])
        src_pid = pid[r_s]

        for c in range(CORES):
            for bi, bnodes in enumerate(bins_per_core[c]):
                w_, jj = bi // BINS_PER_WIN, bi % BINS_PER_WIN
                for q in range(NCHUNK):
                    fill = 0
                    for rank, n in enumerate(bnodes):
                        s0 = node_base[n] + starts[n, q]
                        m = int(flat_cnt[n, q])
                        if m == 0:
                            continue
                        assert fill + m <= BIN_CAP, (c, k, bi, q, fill, m)
                        for seg0, seg1 in _split128(fill, fill + m):
                            h, sa = seg0 // 128, seg0 % 128
                            sb = sa + (seg1 - seg0)
                            blk = q * 16 + jj * TILES_PER_BINCHUNK + h
                            off = s0 + (seg0 - fill)
                            i0 = (jj * TILES_PER_BINCHUNK + h) * 128 + sa
                            idx_all[c, k, w_, q, i0:i0 + sb - sa] = \
                                src_pid[off:off + sb - sa] - q * CHUNK
                            s_all[c, k, w_, sa:sb, blk * BIN_COLS + rank] = \
                                n_s[off:off + sb - sa]
                        fill += m

    P.NW, P.PADN, P.CHUNK, P.TROWS = NW, PADN, CHUNK, TROWS
    P.pid = pid
    P.idx_all = idx_all
    P.s_all = s_all
    P.bins_per_core = bins_per_core
    return P


def _wrap_idx(flat2048):
    """2048 flat indices -> [128, 128] int16 (i -> [i%16, i//16], replicated x8)."""
    w = flat2048.reshape(128, 16).T.astype(np.int16)   # [16, 128]
    return np.tile(w, (8, 1))


def build_inmaps(inputs, P):
    x = np.asarray(inputs["x"]).astype(np.float32)
    W1 = np.asarray(inputs["W1"]).astype(np.float32)
    W2 = np.asarray(inputs["W2"]).astype(np.float32)
    W3 = np.asarray(inputs["W3"]).astype(np.float32)
    Wl = np.asarray(inputs["Wl"]).astype(np.float32)
    bl = np.asarray(inputs["bl"]).astype(np.float32)
    b1 = np.asarray(inputs["b1"]).astype(np.float32).reshape(H)
    b2 = np.asarray(inputs["b2"]).astype(np.float32).reshape(H)
    b3 = np.asarray(inputs["b3"]).astype(np.float32).reshape(H)
    qe = np.asarray(inputs["query_edges"]).astype(np.int64)

    NW, PADN, TROWS = P.NW, P.PADN, P.TROWS
    inv = np.zeros(TROWS, np.int64)   # pid -> node (garbage rows stay 0)
    inv[P.pid] = np.arange(N)
    live = np.zeros(TROWS, bool)
    live[P.pid] = True

    # padded x, transposed: [F_IN, PADN] per core
    x_pad = np.where(live[:, None], x[inv], 0.0).astype(np.float32)  # [TROWS, F_IN]

    # W2/W3 split into per-set blocks: y = sum_k relu_k @ W2k.T
    # W2 [H, 3H]; W2k = W2[:, k*H:(k+1)*H]; lhsT = W2k.T? out = lhsT.T @ rhs
    # want y.T [64, cols] = W2k @ r_k.T fm -> lhsT[f_in, f_out] = W2k.T
    W23 = np.zeros((2, 3, H, H), np.float32)
    for k in range(3):
        W23[0, k] = W2[:, k * H:(k + 1) * H].T
        W23[1, k] = W3[:, k * H:(k + 1) * H].T
    W23_flat = W23.transpose(2, 0, 1, 3).reshape(H, 6 * H)  # [64, (l k q)]
    # Wl: z cols [zA0 zA1 zB0 zB1]; lhsT_k [64, 4]
    WlT = np.zeros((3, H, 4), np.float32)
    WlA, WlB = Wl[:, :3 * H], Wl[:, 3 * H:]
    for k in range(3):
        WlT[k, :, 0:2] = WlA[:, k * H:(k + 1) * H].T
        WlT[k, :, 2:4] = WlB[:, k * H:(k + 1) * H].T
    WlT_flat = WlT.transpose(1, 0, 2).reshape(H, 12)  # [64, (k q)]

    qsrc_pid = P.pid[qe[:, 0]]
    qdst_pid = P.pid[qe[:, 1]]

    in_maps = []
    for c in range(CORES):
        xT = x_pad[c * PADN:(c + 1) * PADN].T.copy()      # [128, PADN]
        idx = np.zeros((3, NW, NCHUNK, 128, 128), np.int16)
        for k in range(3):
            for w in range(NW):
                for q in range(NCHUNK):
                    idx[k, w, q] = _wrap_idx(P.idx_all[c, k, w, q])
        qs = np.zeros((QT * 128,), np.int32)
        qd = np.zeros((QT * 128,), np.int32)
        qs[:QPC] = qsrc_pid[c * QPC:(c + 1) * QPC]
        qd[:QPC] = qdst_pid[c * QPC:(c + 1) * QPC]
        in_maps.append({
            "xT": xT.astype(BF16),
            "idx": idx,
            "S": P.s_all[c].astype(BF16),                  # [3, NW, 128, 2048]
            "W1T": W1.T.astype(BF16),                      # [128, 64]
            "W23T": W23_flat.astype(BF16),                 # [64, 384]
            "WlT": WlT_flat.astype(BF16),                  # [64, 12]
            "biases": np.stack([b1, b2, b3], axis=1).astype(np.float32),  # [64,3]
            "blv": np.tile(bl.reshape(1, 2), (128, 1)).astype(np.float32),
            "qsrc": qs.reshape(QT, 128, 1),
            "qdst": qd.reshape(QT, 128, 1),
        })
    return in_maps


# ------------------------------------------------------------ numpy verifier

def numpy_pipeline(inputs, P, in_maps):
    """Simulate the device dataflow exactly (f32 math) -> [Q, 2] softmax."""
    NW, PADN, CHUNK, TROWS = P.NW, P.PADN, P.CHUNK, P.TROWS
    W1T = np.asarray(in_maps[0]["W1T"], np.float32)
    W23 = np.asarray(in_maps[0]["W23T"], np.float32)
    WlT = np.asarray(in_maps[0]["WlT"], np.float32)
    biases = in_maps[0]["biases"]
    bl = in_maps[0]["blv"]

    # layer-0 table
    table = np.zeros((TROWS, 64), np.float32)
    for c in range(CORES):
        xT = np.asarray(in_maps[c]["xT"], np.float32)
        table[c * PADN:(c + 1) * PADN] = (W1T.T @ xT).T

    for layer in range(3):
        new_table = np.zeros((TROWS, 64), np.float32)
        zt = np.zeros((TROWS, 4), np.float32)
        for c in range(CORES):
            S = np.asarray(in_maps[c]["S"], np.float32)
            for w in range(NW):
                rk = []
                for k in range(3):
                    psum = np.zeros((64, WIN_COLS), np.float32)
                    for q in range(NCHUNK):
                        flat = P.idx_all[c, k, w, q] + q * CHUNK   # [2048]
                        msgs = table[flat]                          # [2048, 64]
                        for jj in range(BINS_PER_WIN):
                            for h in range(TILES_PER_BINCHUNK):
                                t = jj * TILES_PER_BINCHUNK + h
                                b = q * 16 + t
                                lhsT = msgs[t * 128:(t + 1) * 128]
                                rhs = S[k, w, :, b * BIN_COLS:(b + 1) * BIN_COLS]
                                psum[:, jj * BIN_COLS:(jj + 1) * BIN_COLS] += lhsT.T @ rhs
                    rk.append(np.maximum(psum + biases[:, k:k + 1], 0.0))
                if layer < 2:
                    y = np.zeros((64, WIN_COLS), np.float32)
                    for k in range(3):
                        blk = W23[:, (layer * 3 + k) * H:(layer * 3 + k + 1) * H]
                        y += blk.T @ rk[k]
                    new_table[c * PADN + w * WIN_COLS:
                              c * PADN + (w + 1) * WIN_COLS] = y.T
                else:
                    z = np.zeros((4, WIN_COLS), np.float32)
                    for k in range(3):
                        z += WlT[:, k * 4:(k + 1) * 4].T @ rk[k]
                    zt[c * PADN + w * WIN_COLS:
                       c * PADN + (w + 1) * WIN_COLS] = z.T
        table = new_table

    out = np.zeros((Q, 2), np.float32)
    for c in range(CORES):
        qs = in_maps[c]["qsrc"].reshape(-1)
        qd = in_maps[c]["qdst"].reshape(-1)
        logits = zt[qs][:, 0:2] + zt[qd][:, 2:4] + bl
        m = logits.max(1, keepdims=True)
        e = np.exp(logits - m)
        sm = e / e.sum(1, keepdims=True)
        out[c * QPC:(c + 1) * QPC] = sm[:QPC]
    return out


def kernel(x, edge_index, edge_in, edge_out, query_edges, in_w, out_w,
           W1, W2, W3, b1, b2, b3, Wl, bl):
    inputs = dict(x=x, edge_index=edge_index, edge_in=edge_in, edge_out=edge_out,
                  query_edges=query_edges, in_w=in_w, out_w=out_w,
                  W1=W1, W2=W2, W3=W3, b1=b1, b2=b2, b3=b3, Wl=Wl, bl=bl)
    P = pack(inputs)
    in_maps = build_inmaps(inputs, P)
    if os.environ.get("DGCN_NUMPY", "0") == "1":
        return numpy_pipeline(inputs, P, in_maps)
    from dgcn_device import run_device
    out, _res = run_device(P, in_maps)
    return out
